# revision 9
# baseline (speedup 1.0000x reference)
"""DMTet geometry (marching tetrahedra) kernel for 8x Trainium2 NeuronCores.

Strategy:
  - Host (numpy): gather per-tet vertex records [x,y,z,sdf] (fine-grained
    random gather is not supported by the trn2 indirect-DMA path: it consumes
    one index per partition), plus the integer topology (occupancy, valid
    tets, unique-edge dedup, triangle tables).
  - Device (SPMD over 8 cores, tets sharded): stream the dense per-tet
    records and compute the sdf=0 linear interpolation for all 6 edges of
    every tet on the vector engine.  Output is dense [tets, 6, 3] f32.
    This is the memory-regime bulk: 16 f32 in + 18 f32 out per tet,
    ~204 MB streamed across 8 cores.
  - Host: select one interpolated vertex per unique crossing edge.  The
    interpolated value for an edge is bit-identical across duplicate
    instances and endpoint orientations (IEEE negation symmetry), so any
    instance can be selected.
"""

import numpy as np

P = 128
N_V = 300000
F_TOTAL = 1500000
N_CORES = 8
K_TOT = 1470            # tet slots per partition per core
C_CHUNK = 294           # tet slots per partition per pipeline chunk
N_CHUNKS = K_TOT // C_CHUNK
F_PC = F_TOTAL // N_CORES       # 187500 real tets per core
T_PC = P * K_TOT                # 188160 padded tets per core

RECIP_MODE = "dve"      # "dve" (bit-exact 1/x, slower) | "approx" (~2 ulp)

BASE_TET_EDGES = [(0, 1), (0, 2), (0, 3), (1, 2), (1, 3), (2, 3)]
# groups of edges sharing vertex a, with contiguous-in-e layout:
#   e0..2 = (0,{1,2,3}), e3..4 = (1,{2,3}), e5 = (2,{3})
EDGE_GROUPS = [(0, [1, 2, 3], 0), (1, [2, 3], 3), (2, [3], 5)]
BASE_TET_EDGES_FLAT = np.array([0, 1, 0, 2, 0, 3, 1, 2, 1, 3, 2, 3], dtype=np.int64)

TRIANGLE_TABLE = np.array([
    [-1, -1, -1, -1, -1, -1], [1, 0, 2, -1, -1, -1], [4, 0, 3, -1, -1, -1],
    [1, 4, 2, 1, 3, 4], [3, 1, 5, -1, -1, -1], [2, 3, 0, 2, 5, 3],
    [1, 4, 0, 1, 5, 4], [4, 2, 5, -1, -1, -1], [4, 5, 2, -1, -1, -1],
    [4, 1, 0, 4, 5, 1], [3, 2, 0, 3, 5, 2], [1, 3, 5, -1, -1, -1],
    [4, 1, 2, 4, 3, 1], [3, 0, 4, -1, -1, -1], [2, 0, 1, -1, -1, -1],
    [-1, -1, -1, -1, -1, -1]], dtype=np.int64)
NUM_TRIANGLES_TABLE = np.array([0, 1, 1, 2, 1, 2, 2, 1, 1, 2, 2, 1, 2, 1, 1, 0],
                               dtype=np.int64)


def build_bass_kernel(k_tot=K_TOT, c_chunk=C_CHUNK, recip_mode=RECIP_MODE):
    """Per-core bass program: for each tet slot, input 16 f32
    [x,y,z,s] x 4 vertices; output 18 f32 = 6 edges x 3 coords of the
    interpolated sdf=0 vertex.

    Inputs: gin [P, k_tot*16] f32.  Output: verts6 [P, k_tot*18] f32.
    Formula per edge (a, b):
        d = s_a - s_b ; r = clamp(1/d, +-1e30)
        w1 = s_a * r ; w0m = s_b * r
        vert_c = x_b_c * w1 - x_a_c * w0m
    """
    import concourse.bacc as bacc
    import concourse.mybir as mybir
    from concourse.tile import TileContext

    f32 = mybir.dt.float32
    Alu = mybir.AluOpType

    nc = bacc.Bacc(None, target_bir_lowering=False)
    gin = nc.dram_tensor("gin", [P, k_tot * 16], f32, kind="ExternalInput")
    vout = nc.dram_tensor("verts6", [P, k_tot * 18], f32, kind="ExternalOutput")

    n_chunks = k_tot // c_chunk
    assert n_chunks * c_chunk == k_tot
    C = c_chunk

    with TileContext(nc) as tc:
        with (
            tc.tile_pool(name="io", bufs=2) as io,
            tc.tile_pool(name="tmp", bufs=1) as tmp,
        ):
            for it in range(n_chunks):
                gt = io.tile([P, 16 * C], f32, tag="g")
                nc.sync.dma_start(out=gt[:], in_=gin[:, it * 16 * C:(it + 1) * 16 * C])
                g3 = gt[:].rearrange("p (t r) -> p t r", r=16)   # [P, C, 16]

                d = tmp.tile([P, 6 * C], f32, tag="d")
                dv = d[:].rearrange("p (t e) -> p t e", e=6)
                for (a, bs, eo) in EDGE_GROUPS:
                    n = len(bs)
                    sa = g3[:, :, 4 * a + 3].unsqueeze(2).to_broadcast([P, C, n])
                    sb = g3[:, :, 4 * bs[0] + 3::4]
                    nc.vector.tensor_tensor(
                        out=dv[:, :, eo:eo + n], in0=sa, in1=sb, op=Alu.subtract)

                r = tmp.tile([P, 6 * C], f32, tag="r")
                if recip_mode == "approx":
                    scr = tmp.tile([P, 6 * C], f32, tag="scr")
                    nc.vector.reciprocal_approx_accurate(
                        out=r[:], in_=d[:], scratch=scr[:])
                else:
                    nc.vector.reciprocal(out=r[:], in_=d[:])
                # clamp 1/d to +-1e30: identity for surviving edges, keeps
                # degenerate (d == 0) lanes finite.
                rc = tmp.tile([P, 6 * C], f32, tag="rc")
                nc.vector.tensor_scalar(
                    out=rc[:], in0=r[:], scalar1=1e30, scalar2=-1e30,
                    op0=Alu.min, op1=Alu.max)
                rcv = rc[:].rearrange("p (t e) -> p t e", e=6)

                w1 = tmp.tile([P, 6 * C], f32, tag="w1")
                w1v = w1[:].rearrange("p (t e) -> p t e", e=6)
                w0m = tmp.tile([P, 6 * C], f32, tag="w0m")
                w0mv = w0m[:].rearrange("p (t e) -> p t e", e=6)
                for (a, bs, eo) in EDGE_GROUPS:
                    n = len(bs)
                    sa = g3[:, :, 4 * a + 3].unsqueeze(2).to_broadcast([P, C, n])
                    sb = g3[:, :, 4 * bs[0] + 3::4]
                    nc.vector.tensor_tensor(
                        out=w1v[:, :, eo:eo + n], in0=sa,
                        in1=rcv[:, :, eo:eo + n], op=Alu.mult)
                    nc.vector.tensor_tensor(
                        out=w0mv[:, :, eo:eo + n], in0=sb,
                        in1=rcv[:, :, eo:eo + n], op=Alu.mult)

                u = tmp.tile([P, 18 * C], f32, tag="u")
                uv = u[:].rearrange("p (t e c) -> p t e c", e=6, c=3)
                t = tmp.tile([P, 18 * C], f32, tag="t")
                tv = t[:].rearrange("p (t e c) -> p t e c", e=6, c=3)
                for e, (a, b) in enumerate(BASE_TET_EDGES):
                    xa = g3[:, :, 4 * a:4 * a + 3]
                    xb = g3[:, :, 4 * b:4 * b + 3]
                    nc.vector.tensor_tensor(
                        out=uv[:, :, e, :], in0=xa,
                        in1=w0mv[:, :, e].unsqueeze(2).to_broadcast([P, C, 3]),
                        op=Alu.mult)
                    nc.vector.tensor_tensor(
                        out=tv[:, :, e, :], in0=xb,
                        in1=w1v[:, :, e].unsqueeze(2).to_broadcast([P, C, 3]),
                        op=Alu.mult)

                o = io.tile([P, 18 * C], f32, tag="o")
                nc.vector.tensor_sub(out=o[:], in0=t[:], in1=u[:])
                nc.scalar.dma_start(
                    out=vout[:, it * 18 * C:(it + 1) * 18 * C], in_=o[:])
    nc.compile()
    return nc


_NC_CACHE = {}


def _get_nc():
    if "nc" not in _NC_CACHE:
        _NC_CACHE["nc"] = build_bass_kernel()
    return _NC_CACHE["nc"]


def _run_device(gin_per_core):
    """Run the SPMD bass kernel on 8 cores; returns verts6 per core."""
    from concourse.bass_utils import run_bass_kernel_spmd

    nc = _get_nc()
    in_maps = [{"gin": gin_per_core[k]} for k in range(N_CORES)]
    res = run_bass_kernel_spmd(nc, in_maps, core_ids=list(range(N_CORES)))
    return [res.results[k]["verts6"] for k in range(N_CORES)]


def _host_topology(sdf, tet):
    """Mirror of the reference integer topology.  Returns (valid_idx,
    crossing_instance, faces_i64) where crossing_instance[j] indexes the flat
    [T*6] edge-instance list for the j-th unique crossing edge in
    lexicographic order."""
    occ = sdf > 0                                   # [N] bool
    occ4 = occ[tet]                                 # [F, 4]
    occ_sum = occ4.sum(1)
    valid = (occ_sum > 0) & (occ_sum < 4)
    valid_idx = np.nonzero(valid)[0]                # [T]
    tets = tet[valid_idx].astype(np.int64)          # [T, 4]
    edges = tets[:, BASE_TET_EDGES_FLAT].reshape(-1, 2)   # [T*6, 2]
    e_lo = np.minimum(edges[:, 0], edges[:, 1])
    e_hi = np.maximum(edges[:, 0], edges[:, 1])
    key = (e_lo << 19) | e_hi                       # order == lex (lo, hi)
    uniq, inverse = np.unique(key, return_inverse=True)
    inverse = inverse.reshape(-1)
    u_lo = (uniq >> 19)
    u_hi = (uniq & ((1 << 19) - 1))
    mask_edges = occ[u_lo] ^ occ[u_hi]              # exactly one endpoint inside
    mapping = np.full(uniq.shape[0], -1, dtype=np.int64)
    mapping[mask_edges] = np.arange(int(mask_edges.sum()), dtype=np.int64)
    idx_map = mapping[inverse].reshape(-1, 6)
    # one representative instance per unique edge (first occurrence)
    inst = np.empty(uniq.shape[0], dtype=np.int64)
    inst[inverse[::-1]] = np.arange(inverse.shape[0] - 1, -1, -1, dtype=np.int64)
    crossing_inst = inst[mask_edges]
    # faces from triangle tables
    v_id = 1 << np.arange(4, dtype=np.int64)
    tetindex = (occ4[valid_idx].astype(np.int64) * v_id).sum(-1)
    num_tri = NUM_TRIANGLES_TABLE[tetindex]
    tt = TRIANGLE_TABLE[tetindex]
    m1 = num_tri == 1
    m2 = num_tri == 2
    faces1 = np.take_along_axis(idx_map[m1], tt[m1][:, :3], axis=1).reshape(-1, 3)
    faces2 = np.take_along_axis(idx_map[m2], tt[m2][:, :6], axis=1).reshape(-1, 3)
    faces = np.concatenate([faces1, faces2], axis=0)
    return valid_idx, crossing_inst, faces


def kernel(pos_nx3, sdf_n, tet_fx4):
    pos = np.asarray(pos_nx3, dtype=np.float32)
    sdf = np.asarray(sdf_n, dtype=np.float32)
    tet_in = np.asarray(tet_fx4)
    tet = np.ascontiguousarray(tet_in)

    # --- host gather of per-tet vertex records ---
    pv = np.concatenate([pos, sdf[:, None]], axis=1).astype(np.float32)  # [N,4]
    rec = pv[tet]                                   # [F, 4, 4]
    rec = rec.reshape(F_TOTAL, 16)
    pad_rec = pv[[0, 1, 2, 3]].reshape(1, 16)
    gin_per_core = []
    for k in range(N_CORES):
        shard = rec[k * F_PC:(k + 1) * F_PC]
        pad = np.repeat(pad_rec, T_PC - shard.shape[0], axis=0)
        full = np.concatenate([shard, pad], axis=0)      # [T_PC, 16]
        gin_per_core.append(np.ascontiguousarray(full.reshape(P, K_TOT * 16)))

    # --- device: interpolate all 6 edges of every tet ---
    verts6_cores = _run_device(gin_per_core)
    verts6 = np.concatenate(
        [v.reshape(T_PC, 18) for v in verts6_cores], axis=0)  # [8*T_PC, 18]
    verts6 = verts6.reshape(-1, 6, 3)

    # --- host: topology + selection ---
    valid_idx, crossing_inst, faces = _host_topology(sdf, tet)
    f_orig = valid_idx[crossing_inst // 6]            # original tet id
    e_slot = crossing_inst % 6
    row = (f_orig // F_PC) * T_PC + (f_orig % F_PC)   # padded row in verts6
    verts = verts6[row, e_slot, :]

    faces_dtype = np.int64 if tet_in.dtype == np.int64 else np.int32
    return verts.astype(np.float32, copy=False), faces.astype(faces_dtype)
